# revision 16
# baseline (speedup 1.0000x reference)
"""ArcFace combined-margin loss kernel for 8 TRN2 NeuronCores.

Strategy (v4)
-------------
reference: cos = (f @ w.T) / (|f||w|); phi = arcface(cos);
outputs = s*(labels*phi + (1-labels)*cos); loss = mean over rows of
-(sum of log_softmax(outputs) at lab_pinds, masked) / L^2.

The only device-scale work is the dense denominator
sexp[b] = sum_c exp(30*cos[b,c] - 30): per core B*C/8 fp8 MACs (PE,
~33.4us at 157 TF/s DoubleRow, 0.833ns/element) and B*C/8 exps.
Everything else is O(B*L*D + C*D) on host float64.

Per 128-row block (16/core, 2500 classes = 5 chunks x 500, one PSUM
bank each; bank of stream slot s is s%8):

  * 3 chunks -> one ACT Exp with accum_out: the scalar engine's
    accumulator gives the row-sum of this share for free
    (sexp[:, block]); ~1434+283ns/block, under the 2083ns PE block.
  * 2 chunks -> DVE Schraudolph exp, one tensor_scalar over contiguous
    banks (split in two where the rotation wraps): bits =
    int16(psum*A + B) = the bf16 bit pattern of ~exp(arg) (A,B fold
    the /16 scale, -30 bias, log2(e), exponent bias, and a
    mean-error-zeroing offset); ~1282ns/block.  Strips DMA to HBM;
    host sums in f64 (elem err ~2% rms washes out over the sums;
    validated loss rel err ~2e-5 on HW).

Dependency tracking treats strided APs as their bounding range, so
every eviction AP must cover a contiguous, non-wrapping bank run:
where the 5-over-8 bank rotation wraps, the DVE group splits into two
ops (2+1) instead of using a wrapped AP (v3's wrapped groups caused
2-3us false-WAR stalls per occurrence and knocked the PE out of its
2.4GHz p-state).  Input DMAs use chunk-major DRAM layouts so each
piece is 2KB-contiguous per partition (500B strided descriptors made
v3's input take ~15us).  Blocks 0,1 are emitted chunk-interleaved so
wT chunk c is first needed ~2c slots in; dummy fp8 matmuls warm the
PE's DVFS p-state during the DMA lead-in.
Host (numpy float64): positive dots f.w[pinds] exactly, arcface margin,
denominator correction at positives, logsumexp, masked ragged CE, mean.
No collectives (8 partial sexp sets summed on host during unsharding).
"""

import math
import sys

import numpy as np
import ml_dtypes

for _p in ("/opt/trn_rl_repo",):
    if _p not in sys.path:
        sys.path.append(_p)

import concourse.bacc as bacc
import concourse.mybir as mybir
import concourse.tile as tile
from concourse.bass_utils import run_bass_kernel_spmd
from contextlib import ExitStack

B, C, D, LMAX = 2048, 20000, 512, 8
NCORES = 8
CSH = C // NCORES          # 2500 real classes per core
NCH = 5                    # chunks per block (5 PSUM banks)
CW = 500                   # uniform chunk width
NBLK = B // 128            # 16 row blocks
KC = D // 128              # 4 contraction chunks (128 partitions each)
S = 30.0
M_MARGIN = 0.5
FSC = 30.0                 # f rows scaled to 30*unit
WSC = 16.0                 # w rows scaled to 16*unit
# psum dot = FSC*WSC*cos; exp arg = psum/16 - 30 = 30*cos - 30

# Schraudolph bf16-bit exp: bits16 = trunc(psum*A_SCH + B_SCH) is the bf16
# bit pattern of ~exp(psum/16 - 30).  C offset 7.218 zeroes the mean of the
# linear-mantissa error over a uniform fraction; +0.5 centers truncation.
A_SCH = 128.0 * math.log2(math.e) / WSC
B_SCH = 128.0 * (127.0 - S * math.log2(math.e)) - 7.218 + 0.5

F32 = mybir.dt.float32
BF16 = mybir.dt.bfloat16
I16 = mybir.dt.int16
FP8 = mybir.dt.float8e4
E4M3 = ml_dtypes.float8_e4m3

_GRAPH = None


def _schedule():
    """Emission stream + per-block eviction plan.

    emit = [(block, chunk)] in stream order; chunk at slot s lands in
    PSUM bank s%8.  plans[i] = (act_chunks, dve_ops) where act_chunks
    is the accum pair and dve_ops is a list of chunk tuples, each with
    a contiguous (or constant-stride, non-wrapping) bank run.
    """
    emit = []
    for c in range(NCH):
        emit.append((0, c))
        emit.append((1, c))
    for i in range(2, NBLK):
        for c in range(NCH):
            emit.append((i, c))
    banks = {}
    for s, (i, c) in enumerate(emit):
        banks.setdefault(i, {})[c] = s % 8

    plans = {}
    for i in range(NBLK):
        if i < 2:
            # banks stride 2: (0,2,4,6,0) / (1,3,5,7,1)
            act, dve_ops = (0, 1, 2), [(3,), (4,)]
        else:
            off = banks[i][0]
            if off <= 3:
                act, dve_ops = (0, 1, 2), [(3, 4)]       # dve (off+3,off+4)
            elif off == 4:
                act, dve_ops = (0, 1, 2), [(3,), (4,)]   # dve banks (7)+(0)
            elif off == 5:
                act, dve_ops = (0, 1, 2), [(3, 4)]       # dve banks (0,1)
            elif off == 6:
                act, dve_ops = (2, 3, 4), [(0, 1)]       # act (0,1,2), dve (6,7)
            else:  # off == 7: act banks (0,1,2) via chunks 1-3
                act, dve_ops = (1, 2, 3), [(0,), (4,)]   # dve banks (7)+(3)
        plans[i] = (act, dve_ops)
    return emit, banks, plans


def build_graph():
    nc = bacc.Bacc()
    # chunk-major DRAM layouts: every DMA piece is contiguous per
    # partition (2KB lines) so the HW DGE emits fat descriptors.
    fT_ext = nc.declare_dram_parameter("fT8", [8, 128, KC, 256], FP8, isOutput=False)
    wT_ext = nc.declare_dram_parameter("wT8", [NCH, 128, KC, CW], FP8, isOutput=False)
    sexp_ext = nc.declare_dram_parameter("sexp", [128, NBLK], F32, isOutput=True)
    strips_ext = nc.declare_dram_parameter(
        "strips", [NBLK, 128, 2, 512], I16, isOutput=True
    )

    AF = mybir.ActivationFunctionType
    emit, banks, plans = _schedule()

    with ExitStack() as ctx:
        tc = ctx.enter_context(tile.TileContext(nc))
        const = ctx.enter_context(tc.tile_pool(name="const", bufs=1))
        resident = ctx.enter_context(tc.tile_pool(name="resident", bufs=1))
        pmm = ctx.enter_context(tc.tile_pool(name="pmm", bufs=1, space="PSUM"))
        scr = ctx.enter_context(tc.tile_pool(name="scr", bufs=2))

        warm = const.tile([128, 2, 512], FP8)
        nbias = const.tile([128, 1], F32)
        nc.vector.memset(warm[:], 0.0)
        nc.vector.memset(nbias[:], -S)
        dact = const.tile([128, 1], BF16)

        fT = resident.tile([128, 8, KC, 256], FP8)
        wT = resident.tile([128, NCH, KC, CW], FP8)
        strip = resident.tile([128, NBLK, 2, 512], I16)
        sexp_t = resident.tile([128, NBLK], F32)
        psum = pmm.tile([128, 8, 512], F32)

        # Input DMA: pieces ordered by first use across the 3 DMA-capable
        # queues; wT chunk c is first needed at stream slot ~2c, fT piece
        # j (blocks 2j,2j+1) well after the lead-in for j>=1.
        nc.sync.dma_start(wT[:, 0], wT_ext[0])
        nc.gpsimd.dma_start(fT[:, 0], fT_ext[0])
        # dact (Exp table preload, no DMA) leads the scalar queue so only
        # the two first-needed pieces compete for DMA bandwidth early
        nc.scalar.activation(dact[:], nbias[:], AF.Exp, bias=nbias[:], scale=1.0)
        nc.sync.dma_start(wT[:, 1], wT_ext[1])
        nc.gpsimd.dma_start(wT[:, 2], wT_ext[2])
        nc.scalar.dma_start(wT[:, 3], wT_ext[3])
        nc.sync.dma_start(wT[:, 4], wT_ext[4])
        nc.gpsimd.dma_start(fT[:, 1], fT_ext[1])
        nc.sync.dma_start(fT[:, 2], fT_ext[2])
        nc.gpsimd.dma_start(fT[:, 3], fT_ext[3])
        nc.sync.dma_start(fT[:, 4], fT_ext[4])
        nc.gpsimd.dma_start(fT[:, 5], fT_ext[5])
        nc.sync.dma_start(fT[:, 6], fT_ext[6])
        nc.gpsimd.dma_start(fT[:, 7], fT_ext[7])

        # warm up the PE while the input DMAs land: the tensor engine's
        # clock p-state ramps only under SUSTAINED use (0.65 -> 1.2 ->
        # 2.4 GHz after ~3us busy) and drops back when idle, so the
        # warmups bridge the DMA wait and the p-state keeps ramping
        # through the DMA-paced early slots.  Banks 5-7: first real use
        # is stream slot 5.
        for i in range(4):
            nc.tensor.matmul(
                psum[:, 5 + i % 3, :],
                warm[:, :, 0:128],
                warm[:],
                start=True,
                stop=True,
                perf_mode=mybir.MatmulPerfMode.DoubleRow,
            )

        # main stream: 2 DoubleRow matmuls (K=256 each) per chunk into its
        # bank; each eviction fires once its last chunk completes.
        for s, (i, c) in enumerate(emit):
            act, dve_ops = plans[i]
            b = banks[i][c]
            for k2 in range(KC // 2):
                nc.tensor.matmul(
                    psum[:, b, 0:CW],
                    fT[:, i // 2, 2 * k2 : 2 * k2 + 2, 128 * (i % 2) : 128 * (i % 2) + 128],
                    wT[:, c, 2 * k2 : 2 * k2 + 2, :],
                    start=(k2 == 0),
                    stop=(k2 == KC // 2 - 1),
                    perf_mode=mybir.MatmulPerfMode.DoubleRow,
                )
            if c == act[-1]:
                bs = sorted(banks[i][x] for x in act)
                st = bs[1] - bs[0]
                sc = scr.tile([128, 3, CW], BF16, tag="scr")
                nc.scalar.activation(
                    sc[:],
                    psum[:, bs[0] : bs[2] + 1 : st, 0:CW],
                    AF.Exp,
                    bias=nbias[:],
                    scale=1.0 / WSC,
                    accum_out=sexp_t[:, i : i + 1],
                )
            for oi, op in enumerate(dve_ops):
                if c == op[-1]:
                    j0 = sum(len(x) for x in dve_ops[:oi])
                    bs = sorted(banks[i][x] for x in op)
                    g = len(bs)
                    st = (bs[1] - bs[0]) if g > 1 else 1
                    nc.vector.tensor_scalar(
                        strip[:, i, j0 : j0 + g, 0:CW],
                        psum[:, bs[0] : bs[-1] + 1 : st, 0:CW],
                        A_SCH,
                        B_SCH,
                        op0=mybir.AluOpType.mult,
                        op1=mybir.AluOpType.add,
                    )
                    if oi == len(dve_ops) - 1:
                        q = nc.gpsimd if (i % 2 == 0) else nc.sync
                        q.dma_start(strips_ext[i], strip[:, i, :, :])
        nc.scalar.dma_start(sexp_ext[:, :], sexp_t[:, :])

    nc.finalize()
    return nc


def _get_graph():
    global _GRAPH
    if _GRAPH is None:
        _GRAPH = build_graph()
    return _GRAPH


def make_in_maps(f, lab_word2vec, lab_pinds=None):
    f = np.asarray(f, dtype=np.float32)
    w = np.asarray(lab_word2vec, dtype=np.float32)
    fn = np.sqrt((f.astype(np.float64) ** 2).sum(axis=1))
    wn = np.sqrt((w.astype(np.float64) ** 2).sum(axis=1))
    # element (piece, p, k, col) = x[piece*256 + col, k*128+p]
    f8 = (f * (FSC / fn)[:, None].astype(np.float32)).astype(E4M3)
    fT8 = np.ascontiguousarray(
        f8.T.reshape(KC, 128, 8, 256).transpose(2, 1, 0, 3)
    )
    w8 = (w * (WSC / wn)[:, None].astype(np.float32)).astype(E4M3)
    in_maps = []
    for i in range(NCORES):
        wc = w8[i * CSH : (i + 1) * CSH]
        wT8 = np.ascontiguousarray(
            wc.T.reshape(KC, 128, NCH, CW).transpose(2, 1, 0, 3)
        )
        in_maps.append({"fT8": fT8, "wT8": wT8})
    return in_maps


def combine(outs, f, lab_word2vec, lab_pinds, lengths):
    """outs: 8 dicts with sexp [128, NBLK] (ACT accums of 3x500/block)
    and strips [NBLK, 128, 2, 512] int16 (bf16 bit patterns of the
    2x500 DVE share).  Returns float32 loss."""
    f = np.asarray(f, dtype=np.float64)
    w = np.asarray(lab_word2vec, dtype=np.float64)
    pinds = np.asarray(lab_pinds, dtype=np.int64)
    lens = np.asarray(lengths, dtype=np.int64)

    # s_shift[b] = sum_c exp(30 cos - 30); b = i*128 + p
    s_shift = np.zeros(B, dtype=np.float64)
    for o in outs:
        per_block = o["sexp"].astype(np.float64)  # [128, NBLK]
        bits = np.asarray(o["strips"])[:, :, :, 0:CW].view(np.uint16)
        vals = (bits.astype(np.uint32) << 16).view(np.float32)
        dve = vals.astype(np.float64).sum(axis=(2, 3))  # [NBLK, 128]
        s_shift += (per_block + dve.T).T.reshape(B)

    fn = np.sqrt((f * f).sum(axis=1))     # [B]
    wn = np.sqrt((w * w).sum(axis=1))     # [C]
    pd = np.einsum("bjd,bd->bj", w[pinds], f)              # [B, LMAX]
    cos = pd / np.maximum(fn[:, None] * wn[pinds], 1e-8)

    cos_m, sin_m = math.cos(M_MARGIN), math.sin(M_MARGIN)
    th = math.cos(math.pi - M_MARGIN)
    mm = math.sin(math.pi - M_MARGIN) * M_MARGIN
    sine = np.sqrt(np.clip(1.0 - cos * cos, 0.0, 1.0))
    phi = cos * cos_m - sine * sin_m
    phi = np.where(cos > th, phi, cos - mm)

    mask = (np.arange(LMAX)[None, :] < lens[:, None]).astype(np.float64)
    corr = (mask * (np.exp(S * phi - S) - np.exp(S * cos - S))).sum(axis=1)
    z = S + np.log(s_shift + corr)  # logsumexp of outputs, [B]
    pos_sum = (mask * (S * phi)).sum(axis=1)
    L = lens.astype(np.float64)
    per_sample = (L * z - pos_sum) / (L * L)
    return np.float32(per_sample.mean())


def kernel(f, labels, lab_word2vec, lab_pinds, lengths):
    nc = _get_graph()
    in_maps = make_in_maps(f, lab_word2vec)
    res = run_bass_kernel_spmd(nc, in_maps, core_ids=list(range(NCORES)))
    return combine(res.results, f, lab_word2vec, lab_pinds, lengths)


# revision 24
# speedup vs baseline: 1.0666x; 1.0666x over previous
"""ArcFace combined-margin loss kernel for 8 TRN2 NeuronCores.

Strategy (v4)
-------------
reference: cos = (f @ w.T) / (|f||w|); phi = arcface(cos);
outputs = s*(labels*phi + (1-labels)*cos); loss = mean over rows of
-(sum of log_softmax(outputs) at lab_pinds, masked) / L^2.

The only device-scale work is the dense denominator
sexp[b] = sum_c exp(30*cos[b,c] - 30): per core B*C/8 fp8 MACs (PE,
~33.4us at 157 TF/s DoubleRow, 0.833ns/element) and B*C/8 exps.
Everything else is O(B*L*D + C*D) on host float64.

Per 128-row block (16/core, 2500 classes = 5 chunks x 500, one PSUM
bank each; bank of stream slot s is s%8):

  * 3 chunks -> one ACT Exp with accum_out: the scalar engine's
    accumulator gives the row-sum of this share for free
    (sexp[:, block]); ~1434+283ns/block, under the 2083ns PE block.
  * 2 chunks -> DVE Schraudolph exp, one tensor_scalar over contiguous
    banks (split in two where the rotation wraps): bits =
    int16(psum*A + B) = the bf16 bit pattern of ~exp(arg) (A,B fold
    the /16 scale, -30 bias, log2(e), exponent bias, and a
    mean-error-zeroing offset); ~1282ns/block.  Strips DMA to HBM;
    host sums in f64 (elem err ~2% rms washes out over the sums;
    validated loss rel err ~2e-5 on HW).

Dependency tracking treats strided APs as their bounding range, so
every eviction AP must cover a contiguous, non-wrapping bank run:
where the 5-over-8 bank rotation wraps, the DVE group splits into two
ops (2+1) instead of using a wrapped AP (v3's wrapped groups caused
2-3us false-WAR stalls per occurrence and knocked the PE out of its
2.4GHz p-state).  Input DMAs use chunk-major DRAM layouts so each
piece is 2KB-contiguous per partition (500B strided descriptors made
v3's input take ~15us).  Blocks 0,1 are emitted chunk-interleaved so
wT chunk c is first needed ~2c slots in; dummy fp8 matmuls warm the
PE's DVFS p-state during the DMA lead-in.
Host (numpy float64): positive dots f.w[pinds] exactly, arcface margin,
denominator correction at positives, logsumexp, masked ragged CE, mean.
No collectives (8 partial sexp sets summed on host during unsharding).
"""

import math
import sys

import numpy as np
import ml_dtypes

for _p in ("/opt/trn_rl_repo",):
    if _p not in sys.path:
        sys.path.append(_p)

import concourse.bacc as bacc
import concourse.mybir as mybir
import concourse.tile as tile
from concourse.bass_utils import run_bass_kernel_spmd
from contextlib import ExitStack

B, C, D, LMAX = 2048, 20000, 512, 8
NCORES = 8
CSH = C // NCORES          # 2500 real classes per core
NCH = 5                    # chunks per block (5 PSUM banks)
CW = 500                   # uniform chunk width
NBLK = B // 128            # 16 row blocks
KC = D // 128              # 4 contraction chunks (128 partitions each)
S = 30.0
M_MARGIN = 0.5
FSC = 30.0                 # f rows scaled to 30*unit
WSC = 16.0                 # w rows scaled to 16*unit
# psum dot = FSC*WSC*cos; exp arg = psum/16 - 30 = 30*cos - 30

# Schraudolph bf16-bit exp: bits16 = trunc(psum*A_SCH + B_SCH) is the bf16
# bit pattern of ~exp(psum/16 - 30).  C offset 7.218 zeroes the mean of the
# linear-mantissa error over a uniform fraction; +0.5 centers truncation.
A_SCH = 128.0 * math.log2(math.e) / WSC
B_SCH = 128.0 * (127.0 - S * math.log2(math.e)) - 7.218 + 0.5

F32 = mybir.dt.float32
BF16 = mybir.dt.bfloat16
I16 = mybir.dt.int16
FP8 = mybir.dt.float8e4
E4M3 = ml_dtypes.float8_e4m3

_GRAPH = None


def _schedule():
    """Emission stream + per-block eviction plan.

    emit = [(block, chunk)] in stream order; chunk at slot s lands in
    PSUM bank s%8.  plans[i] = (act_chunks, dve_ops) where act_chunks
    is the accum pair and dve_ops is a list of chunk tuples, each with
    a contiguous (or constant-stride, non-wrapping) bank run.
    """
    emit = []
    for c in range(NCH):
        emit.append((0, c))
        emit.append((1, c))
    for i in range(2, NBLK):
        for c in range(NCH):
            emit.append((i, c))
    banks = {}
    for s, (i, c) in enumerate(emit):
        banks.setdefault(i, {})[c] = s % 8

    plans = {}
    for i in range(NBLK):
        off = banks[i][0]
        if i < 2:
            # banks stride 2: (0,2,4,6,0) / (1,3,5,7,1)
            act, dve_ops = (0, 1), [(2, 3), (4,)]
        elif i < 8:
            # ACT-2 / DVE-3: extra eviction slack while the input DMAs
            # and p-state ramp still perturb the pipeline
            if off <= 3:
                act, dve_ops = (0, 1), [(2, 3, 4)]
            elif off == 4:
                act, dve_ops = (0, 1), [(2, 3), (4,)]    # dve banks (6,7)+(0)
            elif off == 5:
                act, dve_ops = (0, 1), [(2,), (3, 4)]    # (7)+(0,1)
            elif off == 6:
                act, dve_ops = (0, 1), [(2, 3, 4)]       # dve banks (0,1,2)
            else:  # off == 7
                act, dve_ops = (1, 2), [(0,), (3, 4)]    # act (0,1), dve (7)+(2,3)
        else:
            # ACT-3 / DVE-2: smaller strips toward the tail
            if off <= 3:
                act, dve_ops = (0, 1, 2), [(3, 4)]       # dve (off+3,off+4)
            elif off == 4:
                act, dve_ops = (0, 1, 2), [(3,), (4,)]   # dve banks (7)+(0)
            elif off == 5:
                act, dve_ops = (0, 1, 2), [(3, 4)]       # dve banks (0,1)
            elif off == 6:
                act, dve_ops = (2, 3, 4), [(0, 1)]       # act (0,1,2), dve (6,7)
            else:  # off == 7: act banks (0,1,2) via chunks 1-3
                act, dve_ops = (1, 2, 3), [(0,), (4,)]   # dve banks (7)+(3)
        plans[i] = (act, dve_ops)
    return emit, banks, plans


def build_graph():
    nc = bacc.Bacc()
    # chunk-major DRAM layouts: every DMA piece is contiguous per
    # partition (2KB lines) so the HW DGE emits fat descriptors.
    fT_ext = nc.declare_dram_parameter("fT8", [8, 128, KC, 256], FP8, isOutput=False)
    wT_ext = nc.declare_dram_parameter("wT8", [NCH, 128, KC, CW], FP8, isOutput=False)
    sexp_ext = nc.declare_dram_parameter("sexp", [128, NBLK], F32, isOutput=True)
    strips_ext = nc.declare_dram_parameter(
        "strips", [NBLK, 128, 3, 512], I16, isOutput=True
    )

    AF = mybir.ActivationFunctionType
    emit, banks, plans = _schedule()

    with ExitStack() as ctx:
        tc = ctx.enter_context(tile.TileContext(nc))
        const = ctx.enter_context(tc.tile_pool(name="const", bufs=1))
        resident = ctx.enter_context(tc.tile_pool(name="resident", bufs=1))
        pmm = ctx.enter_context(tc.tile_pool(name="pmm", bufs=1, space="PSUM"))
        scr = ctx.enter_context(tc.tile_pool(name="scr", bufs=2))

        warm = const.tile([128, 2, 512], FP8)
        nbias = const.tile([128, 1], F32)
        nc.vector.memset(warm[:], 0.0)
        nc.vector.memset(nbias[:], -S)
        dact = const.tile([128, 1], BF16)

        fT = resident.tile([128, 8, KC, 256], FP8)
        wT = resident.tile([128, NCH, KC, CW], FP8)
        strip = resident.tile([128, NBLK, 3, 512], I16)
        sexp_t = resident.tile([128, NBLK], F32)
        psum = pmm.tile([128, 8, 512], F32)

        # Input DMA: pieces ordered by first use across the 3 DMA-capable
        # queues; wT chunk c is first needed at stream slot ~2c, fT piece
        # j (blocks 2j,2j+1) well after the lead-in for j>=1.
        nc.sync.dma_start(wT[:, 0], wT_ext[0])
        nc.gpsimd.dma_start(fT[:, 0], fT_ext[0])
        nc.scalar.dma_start(wT[:, 1], wT_ext[1])
        nc.sync.dma_start(wT[:, 2], wT_ext[2])
        nc.gpsimd.dma_start(wT[:, 3], wT_ext[3])
        nc.scalar.dma_start(wT[:, 4], wT_ext[4])
        nc.sync.dma_start(fT[:, 1], fT_ext[1])
        nc.gpsimd.dma_start(fT[:, 2], fT_ext[2])
        nc.sync.dma_start(fT[:, 3], fT_ext[3])
        nc.gpsimd.dma_start(fT[:, 4], fT_ext[4])
        nc.sync.dma_start(fT[:, 5], fT_ext[5])
        nc.gpsimd.dma_start(fT[:, 6], fT_ext[6])
        nc.sync.dma_start(fT[:, 7], fT_ext[7])
        # preload the Exp activation table off the critical path
        nc.scalar.activation(dact[:], nbias[:], AF.Exp, bias=nbias[:], scale=1.0)

        # warm up the PE while the input DMAs land: the tensor engine's
        # clock p-state ramps only under SUSTAINED use (0.65 -> 1.2 ->
        # 2.4 GHz after ~3us busy) and drops back when idle, so the
        # warmups bridge the whole DMA wait (~3.5us) and the first real
        # matmuls start at full clock.  Banks 5-7: first real use is
        # stream slot 5.
        for i in range(8):
            nc.tensor.matmul(
                psum[:, 5 + i % 3, :],
                warm[:, :, 0:128],
                warm[:],
                start=True,
                stop=True,
                perf_mode=mybir.MatmulPerfMode.DoubleRow,
            )

        # main stream: 2 DoubleRow matmuls (K=256 each) per chunk into its
        # bank; each eviction fires once its last chunk completes.
        for s, (i, c) in enumerate(emit):
            act, dve_ops = plans[i]
            b = banks[i][c]
            for k2 in range(KC // 2):
                nc.tensor.matmul(
                    psum[:, b, 0:CW],
                    fT[:, i // 2, 2 * k2 : 2 * k2 + 2, 128 * (i % 2) : 128 * (i % 2) + 128],
                    wT[:, c, 2 * k2 : 2 * k2 + 2, :],
                    start=(k2 == 0),
                    stop=(k2 == KC // 2 - 1),
                    perf_mode=mybir.MatmulPerfMode.DoubleRow,
                )
            if c == act[-1]:
                bs = sorted(banks[i][x] for x in act)
                st = bs[1] - bs[0]
                sc = scr.tile([128, 3, CW], BF16, tag="scr")
                nc.scalar.activation(
                    sc[:, : len(bs), :],
                    psum[:, bs[0] : bs[-1] + 1 : st, 0:CW],
                    AF.Exp,
                    bias=nbias[:],
                    scale=1.0 / WSC,
                    accum_out=sexp_t[:, i : i + 1],
                )
            for oi, op in enumerate(dve_ops):
                if c == op[-1]:
                    j0 = sum(len(x) for x in dve_ops[:oi])
                    bs = sorted(banks[i][x] for x in op)
                    g = len(bs)
                    st = (bs[1] - bs[0]) if g > 1 else 1
                    nc.vector.tensor_scalar(
                        strip[:, i, j0 : j0 + g, 0:CW],
                        psum[:, bs[0] : bs[-1] + 1 : st, 0:CW],
                        A_SCH,
                        B_SCH,
                        op0=mybir.AluOpType.mult,
                        op1=mybir.AluOpType.add,
                    )
                    if oi == len(dve_ops) - 1:
                        nd = sum(len(x) for x in dve_ops)
                        q = nc.gpsimd if (i % 2 == 0) else nc.sync
                        q.dma_start(
                            strips_ext[i][:, 0:nd, :], strip[:, i, 0:nd, :]
                        )
        nc.scalar.dma_start(sexp_ext[:, :], sexp_t[:, :])

    nc.finalize()
    return nc


def _get_graph():
    global _GRAPH
    if _GRAPH is None:
        _GRAPH = build_graph()
    return _GRAPH


def make_in_maps(f, lab_word2vec, lab_pinds=None):
    f = np.asarray(f, dtype=np.float32)
    w = np.asarray(lab_word2vec, dtype=np.float32)
    fn = np.sqrt((f.astype(np.float64) ** 2).sum(axis=1))
    wn = np.sqrt((w.astype(np.float64) ** 2).sum(axis=1))
    # element (piece, p, k, col) = x[piece*256 + col, k*128+p]
    f8 = (f * (FSC / fn)[:, None].astype(np.float32)).astype(E4M3)
    fT8 = np.ascontiguousarray(
        f8.T.reshape(KC, 128, 8, 256).transpose(2, 1, 0, 3)
    )
    w8 = (w * (WSC / wn)[:, None].astype(np.float32)).astype(E4M3)
    in_maps = []
    for i in range(NCORES):
        wc = w8[i * CSH : (i + 1) * CSH]
        wT8 = np.ascontiguousarray(
            wc.T.reshape(KC, 128, NCH, CW).transpose(2, 1, 0, 3)
        )
        in_maps.append({"fT8": fT8, "wT8": wT8})
    return in_maps


def combine(outs, f, lab_word2vec, lab_pinds, lengths):
    """outs: 8 dicts with sexp [128, NBLK] (ACT accum share) and strips
    [NBLK, 128, 3, 512] int16 (bf16 bit patterns of the DVE share:
    3 chunks for blocks 0-7, 2 for blocks 8-15).  Returns f32 loss."""
    f = np.asarray(f, dtype=np.float64)
    w = np.asarray(lab_word2vec, dtype=np.float64)
    pinds = np.asarray(lab_pinds, dtype=np.int64)
    lens = np.asarray(lengths, dtype=np.int64)

    # s_shift[b] = sum_c exp(30 cos - 30); b = i*128 + p
    nd = np.array([3 if i < 8 else 2 for i in range(NBLK)])
    s_shift = np.zeros(B, dtype=np.float64)
    for o in outs:
        per_block = o["sexp"].astype(np.float64)  # [128, NBLK]
        bits = np.asarray(o["strips"])[:, :, :, 0:CW].view(np.uint16)
        vals = (bits.astype(np.uint32) << 16).view(np.float32).astype(np.float64)
        vals *= (np.arange(3)[None, None, :, None] < nd[:, None, None, None])
        dve = vals.sum(axis=(2, 3))  # [NBLK, 128]
        s_shift += (per_block + dve.T).T.reshape(B)

    fn = np.sqrt((f * f).sum(axis=1))     # [B]
    wn = np.sqrt((w * w).sum(axis=1))     # [C]
    pd = np.einsum("bjd,bd->bj", w[pinds], f)              # [B, LMAX]
    cos = pd / np.maximum(fn[:, None] * wn[pinds], 1e-8)

    cos_m, sin_m = math.cos(M_MARGIN), math.sin(M_MARGIN)
    th = math.cos(math.pi - M_MARGIN)
    mm = math.sin(math.pi - M_MARGIN) * M_MARGIN
    sine = np.sqrt(np.clip(1.0 - cos * cos, 0.0, 1.0))
    phi = cos * cos_m - sine * sin_m
    phi = np.where(cos > th, phi, cos - mm)

    mask = (np.arange(LMAX)[None, :] < lens[:, None]).astype(np.float64)
    corr = (mask * (np.exp(S * phi - S) - np.exp(S * cos - S))).sum(axis=1)
    z = S + np.log(s_shift + corr)  # logsumexp of outputs, [B]
    pos_sum = (mask * (S * phi)).sum(axis=1)
    L = lens.astype(np.float64)
    per_sample = (L * z - pos_sum) / (L * L)
    return np.float32(per_sample.mean())


def kernel(f, labels, lab_word2vec, lab_pinds, lengths):
    nc = _get_graph()
    in_maps = make_in_maps(f, lab_word2vec)
    res = run_bass_kernel_spmd(nc, in_maps, core_ids=list(range(NCORES)))
    return combine(res.results, f, lab_word2vec, lab_pinds, lengths)
